# revision 28
# baseline (speedup 1.0000x reference)
"""Trainium2 Bass kernel for nn_MultiHeadAttention_867583393876.

Math (per batch b, head h, all matrices 512x512):
  Qm = x[b] @ WQ[h]; Km = x[b] @ WK[h]; Vm = x[b] @ WV[h]
  S  = Qm @ Km                      (the reference's K.reshape is an identity
                                     on a square matrix, so S = Q @ K, not Q@K^T)
  A  = softmax(S / sqrt(512), axis over the QUERY index t (rows of S))
  Zm = A @ Vm
  out[b] = Z.reshape(512, 4096) @ WO   with Z stacked (h, t, e) -> row-scramble:
      out row t' = h*64 + t//8 uses Z_h rows t = 8*(t'%64)+j, j in [0,8).

Key structural fact: head h only feeds output rows [64h, 64h+64) -- the "WO
projection" concatenates head contributions, it does not sum them.  So the
kernel is sharded head-parallel across the 8 cores with NO collectives: core c
computes out[:, 64c:64(c+1), :] for all 16 batches.

Everything is computed in transposed space (partition = channel) so the
softmax reduction runs along the free axis:
  XT = x[b]^T (via PE transpose), QmT = WQ^T @ XT, Km/Vm natural = XT^T @ W,
  ST = Km^T-contract (lhsT=Km natural) vs QmT, softmax per partition row,
  ZmT = lhsT(Vm natural) @ AT, and the WO stage consumes stride-8 free-dim
  slices of ZmT (which exactly realizes the reference's reshape scramble).

All matmuls run in float32r (13-ish mantissa bits, 4x faster than fp32 on the
PE); inputs are rounded to f32r via copy ops as the BIR verifier requires.
"""

import os

import numpy as np

B, T, E, H = 16, 512, 512, 8
N_CORES = 8
SCALE = 1.0 / 22.627416997969522  # 1/sqrt(512)

_CACHE = {}


def _emit(ctx, nc, tc, tile, mybir, aps, precise):
    import concourse.bass as bass
    from concourse.masks import make_identity

    f32 = mybir.dt.float32
    f32r = mybir.dt.float32r
    bf16 = mybir.dt.bfloat16
    x, wq, wk, wv, wo, outp = (
        aps["x"], aps["wq"], aps["wk"], aps["wv"], aps["wo"], aps["out"],
    )
    ts = bass.ts

    def pool(name, bufs, space="SBUF"):
        return ctx.enter_context(tc.tile_pool(name=name, bufs=bufs, space=space))

    # SBUF pools (sizes are KB/partition; SBUF is 224KB/partition)
    wb = 1 if precise else 2
    p_wo = pool("wo", 1)          # 64KB  WO in f32r, resident
    p_w = pool("w", 1)            # 24KB  WQ/WK/WV[h] f32r, resident
    p_stage = pool("stage", 2)    # 4KB  fp32 DMA staging (W, WO)
    p_xn = pool("xn", wb)         # 16KB x[b]^T fp32 staging
    p_xt = pool("xt", wb)         # 16KB  XT f32r
    p_q = pool("q", wb)           # 16KB  QmT f32r
    p_k = pool("k", wb)           # 16KB  Km natural f32r
    p_v = pool("v", wb)           # 16KB  Vm natural f32r
    p_at = pool("at", 1)          # 8KB   AT f32r
    p_scr = pool("scr", 1)        # 2KB   exp scratch fp32
    p_zt = pool("zt", 1)          # 16KB  ZmT f32r for a batch pair (b even+odd)
    p_out = pool("ostage", 1)     # 2KB   output staging
    p_small = pool("small", 4)    # small per-partition scalars
    if precise:
        # residual (error-free-transformation) tiles for the Q/K/S chain
        p_we = pool("we", 1)      # 16KB  WQ/WK residuals f32r
        p_xe = pool("xe", 1)      # 8KB   XT residual f32r
        p_qe = pool("qe", 1)      # 8KB   QmT residual f32r
        p_ke = pool("ke", 1)      # 8KB   Km residual f32r

    ps_mm = pool("ps_mm", 4, space="PSUM")
    ps_st = pool("ps_st", 3, space="PSUM")
    ps_wo = pool("ps_wo", 1, space="PSUM")

    sub = mybir.AluOpType.subtract
    xload = {}

    # ---- load + round weights (once) ----
    def load_round(dram_rows, n_tiles, dst_tile, scope, resid_tile=None, i0=0):
        # dram_rows: function i -> DRAM AP of rows (128, 512)
        with nc.named_scope(scope):
            for i in range(i0, i0 + n_tiles):
                st = p_stage.tile([128, 512], f32, tag="stage")
                nc.sync.dma_start(st[:], dram_rows(i))
                if dst_tile is wo_r:
                    nc.vector.tensor_copy(dst_tile[:, ts(i, 512)], st[:])
                else:
                    nc.any.tensor_copy(dst_tile[:, ts(i, 512)], st[:])
                if resid_tile is not None:
                    nc.vector.tensor_tensor(
                        resid_tile[:, ts(i, 512)], st[:],
                        dst_tile[:, ts(i, 512)].bitcast(f32), op=sub,
                    )

    # x[b]^T load (pre-transposed on host) + round to f32r; called one batch
    # ahead so the DMA + rounding hide under the previous batch's compute.
    def load_x(bb):
        xn = p_xn.tile([128, 4 * 512], f32, tag="xn")
        for i in range(4):
            nc.sync.dma_start(xn[:, ts(i, 512)], x[bb, i * 128:(i + 1) * 128, :])
        xt = p_xt.tile([128, 4 * 512], f32r, tag="xt")
        for i in range(4):
            nc.any.tensor_copy(xt[:, ts(i, 512)], xn[:, ts(i, 512)])
        xload[bb] = (xn, xt)

    wo_r = p_wo.tile([128, 32 * 512], bf16, tag="wo")
    wq_e = wk_e = None
    if precise:
        wq_e = p_we.tile([128, 4 * 512], f32r, tag="wqe")
        wk_e = p_we.tile([128, 4 * 512], f32r, tag="wke")
    wq_r = p_w.tile([128, 4 * 512], f32r, tag="wq")
    load_round(lambda i: wq[i * 128:(i + 1) * 128, :], 4, wq_r, "load_wq", wq_e)
    load_x(0)
    wk_r = p_w.tile([128, 4 * 512], f32r, tag="wk")
    load_round(lambda i: wk[i * 128:(i + 1) * 128, :], 4, wk_r, "load_wk", wk_e)
    wv_r = p_w.tile([128, 4 * 512], f32r, tag="wv")
    load_round(lambda i: wv[i * 128:(i + 1) * 128, :], 4, wv_r, "load_wv")

    zt_state = [None]
    pending = []

    def emit_z(b, vm, at):
        # ---- ZmT = lhsT(Vm natural) @ AT ----
        # The PSUM->SBUF rounding copy scatters straight into the WO-ready
        # scrambled layout: ZS free index = vblk*1024 + j*128 + (half*64+q)
        # where the Z column t = 8q + j and half = b%2.  The WO-stage lhsT
        # is then a plain contiguous 128-wide slice per (j, vblk) k-tile.
        if b % 2 == 0:
            ztp = p_zt.tile([128, 2 * 4 * 512], bf16, tag="zt")
            zt_state[0] = ztp
        zt = zt_state[0]
        half = b % 2
        for vblk in range(4):
            pz = ps_mm.tile([128, 512], f32, tag="mm")
            for m in range(4):
                nc.tensor.matmul(
                    pz[:],
                    vm[:, m * 512 + vblk * 128: m * 512 + vblk * 128 + 128],
                    at[:, ts(m, 512)],
                    start=(m == 0), stop=(m == 3),
                )
            zf = zt[:]
            pf = pz[:]
            dst = bass.AP(
                zf.tensor, zf.offset + vblk * 1024 + half * 64,
                [list(zf.ap[0]), [1, 64], [128, 8]],
            )
            src = bass.AP(
                pf.tensor, pf.offset, [list(pf.ap[0]), [8, 64], [1, 8]]
            )
            nc.vector.tensor_copy(dst, src)

        # ---- WO stage for the (b-1, b) pair ----
        # lhsT free dims: output partitions 0:64 = batch b-1 rows,
        # 64:128 = batch b rows -- a full M=128 matmul per k-tile.
        if b % 2 == 1:
            po = ps_wo.tile([128, 512], f32, tag="wops")
            for kt in range(32):
                j, dblk = kt // 4, kt % 4
                rhs = wo_r[:, ts(kt, 512)]
                lhs = zt[:, dblk * 1024 + j * 128: dblk * 1024 + (j + 1) * 128]
                nc.tensor.matmul(
                    po[:], lhs, rhs,
                    start=(kt == 0), stop=(kt == 31),
                )
            so = p_out.tile([128, 512], f32, tag="so")
            nc.any.tensor_copy(so[:], po[:])
            nc.sync.dma_start(outp[b - 1], so[0:64, :])
            nc.sync.dma_start(outp[b], so[64:128, :])

    load_x(0)

    for b in range(B):
        if b in (0, 1):
            # all 32 WO tiles must be resident before the first WO stage
            # (end of batch 1); split the load to spread the rounding work
            load_round(lambda i: wo[i * 128:(i + 1) * 128, :], 16, wo_r,
                       f"load_wo{b}", i0=16 * b)
        with nc.named_scope(f"batch{b}"):
            if b + 1 < B:
                load_x(b + 1)
            xn, xt = xload.pop(b)
            xe = None
            if precise:
                xe = p_xe.tile([128, 4 * 512], f32r, tag="xe")
                for i in range(4):
                    nc.vector.tensor_tensor(
                        xe[:, ts(i, 512)], xn[:, ts(i, 512)],
                        xt[:, ts(i, 512)].bitcast(f32), op=sub,
                    )

            def mm_acc(ps_tile, pairs):
                for i, (l, r) in enumerate(pairs):
                    nc.tensor.matmul(
                        ps_tile[:], l, r,
                        start=(i == 0), stop=(i == len(pairs) - 1),
                    )

            def col(w, k, blk):
                return w[:, k * 512 + blk * 128: k * 512 + blk * 128 + 128]

            # ---- QmT = WQ^T @ XT ----
            qt = p_q.tile([128, 4 * 512], f32r, tag="q")
            qe = None
            if precise:
                qe = p_qe.tile([128, 4 * 512], f32r, tag="qe")
            for dblk in range(4):
                pq = ps_mm.tile([128, 512], f32, tag="mm")
                pairs = [(col(wq_r, k, dblk), xt[:, ts(k, 512)]) for k in range(4)]
                if precise:
                    pairs += [(col(wq_r, k, dblk), xe[:, ts(k, 512)]) for k in range(4)]
                    pairs += [(col(wq_e, k, dblk), xt[:, ts(k, 512)]) for k in range(4)]
                mm_acc(pq, pairs)
                # fold the 1/sqrt(512) softmax scale into Q's rounding copy so
                # the ST matmul directly produces scaled logits
                nc.any.tensor_scalar_mul(qt[:, ts(dblk, 512)], pq[:], SCALE)
                if precise:
                    nc.vector.scalar_tensor_tensor(
                        qe[:, ts(dblk, 512)], pq[:], SCALE,
                        qt[:, ts(dblk, 512)].bitcast(f32),
                        op0=mybir.AluOpType.mult, op1=sub,
                    )

            # ---- Km natural = XT^T-contract @ WK ----
            km = p_k.tile([128, 4 * 512], f32r, tag="k")
            ke = None
            if precise:
                ke = p_ke.tile([128, 4 * 512], f32r, tag="ke")
            for tblk in range(4):
                pk = ps_mm.tile([128, 512], f32, tag="mm")
                pairs = [(col(xt, k, tblk), wk_r[:, ts(k, 512)]) for k in range(4)]
                if precise:
                    pairs += [(col(xe, k, tblk), wk_r[:, ts(k, 512)]) for k in range(4)]
                    pairs += [(col(xt, k, tblk), wk_e[:, ts(k, 512)]) for k in range(4)]
                mm_acc(pk, pairs)
                nc.any.tensor_copy(km[:, ts(tblk, 512)], pk[:])
                if precise:
                    nc.vector.tensor_tensor(
                        ke[:, ts(tblk, 512)], pk[:],
                        km[:, ts(tblk, 512)].bitcast(f32), op=sub,
                    )

            # ---- Vm natural = XT^T-contract @ WV ----
            vm = p_v.tile([128, 4 * 512], f32r, tag="v")
            for tblk in range(4):
                pv = ps_mm.tile([128, 512], f32, tag="mm")
                mm_acc(pv, [(col(xt, k, tblk), wv_r[:, ts(k, 512)]) for k in range(4)])
                nc.any.tensor_copy(vm[:, ts(tblk, 512)], pv[:])

            # defer Z + WO of the previous batch to here: its softmax
            # inputs are a full batch old, so the PE never stalls on them,
            # and its matmuls cover the latency of this batch's Q/K rounding
            if pending:
                emit_z(*pending.pop())

            # ---- ST = Km^T-contract @ QmT, then softmax along free axis ----
            at = p_at.tile([128, 4 * 512], f32r, tag="at")
            for sblk in range(4):
                pst = ps_st.tile([128, 512], f32, tag="st")
                pairs = [(col(km, m, sblk), qt[:, ts(m, 512)]) for m in range(4)]
                if precise:
                    pairs += [(col(ke, m, sblk), qt[:, ts(m, 512)]) for m in range(4)]
                    pairs += [(col(km, m, sblk), qe[:, ts(m, 512)]) for m in range(4)]
                mm_acc(pst, pairs)
                nmx = p_small.tile([128, 1], f32, tag="nmx")
                nc.vector.tensor_reduce(
                    nmx[:], pst[:], axis=mybir.AxisListType.X,
                    op=mybir.AluOpType.max, negate=True,
                )
                scr = p_scr.tile([128, 512], f32, tag="scr")
                sm = p_small.tile([128, 1], f32, tag="sm")
                nc.scalar.activation(
                    scr[:], pst[:], mybir.ActivationFunctionType.Exp,
                    bias=nmx[:], scale=1.0, accum_out=sm[:],
                )
                rc = p_small.tile([128, 1], f32, tag="rc")
                nc.vector.reciprocal(rc[:], sm[:])
                nc.vector.tensor_scalar_mul(at[:, ts(sblk, 512)], scr[:], rc[:])

            pending.append((b, vm, at))

    emit_z(*pending.pop())


def _build(precise):
    import concourse.bass as bass  # noqa: F401
    import concourse.tile as tile
    from concourse import bacc, mybir

    nc = bacc.Bacc(
        "TRN2",
        target_bir_lowering=False,
        debug=False,
        enable_asserts=False,
        num_devices=N_CORES,
    )
    f32 = mybir.dt.float32
    aps = {
        "x": nc.dram_tensor("x", (B, E, T), f32, kind="ExternalInput").ap(),
        "wq": nc.dram_tensor("wq", (E, E), f32, kind="ExternalInput").ap(),
        "wk": nc.dram_tensor("wk", (E, E), f32, kind="ExternalInput").ap(),
        "wv": nc.dram_tensor("wv", (E, E), f32, kind="ExternalInput").ap(),
        "wo": nc.dram_tensor("wo", (H * E, E), f32, kind="ExternalInput").ap(),
        "out": nc.dram_tensor("out", (B, 64, E), f32, kind="ExternalOutput").ap(),
    }
    from contextlib import ExitStack

    with tile.TileContext(nc) as tc, ExitStack() as ctx:
        _emit(ctx, nc, tc, tile, mybir, aps, precise)
    nc.compile()
    return nc


DEFAULT_PRECISE = False


def _get_nc(precise=None):
    if precise is None:
        precise = DEFAULT_PRECISE
    key = ("nc", bool(precise))
    if key not in _CACHE:
        _CACHE[key] = _build(precise)
    return _CACHE[key]


def run(inputs, trace=False, precise=None):
    from concourse.bass_utils import run_bass_kernel_spmd

    nc = _get_nc(precise)
    x = np.asarray(inputs["x"], dtype=np.float32)
    xT = np.ascontiguousarray(x.transpose(0, 2, 1))
    WQ = np.asarray(inputs["WQ"], dtype=np.float32)
    WK = np.asarray(inputs["WK"], dtype=np.float32)
    WV = np.asarray(inputs["WV"], dtype=np.float32)
    WO = np.ascontiguousarray(np.asarray(inputs["WO"], dtype=np.float32))
    in_maps = [
        {
            "x": xT,
            "wq": np.ascontiguousarray(WQ[c]),
            "wk": np.ascontiguousarray(WK[c]),
            "wv": np.ascontiguousarray(WV[c]),
            "wo": WO,
        }
        for c in range(N_CORES)
    ]
    res = run_bass_kernel_spmd(
        nc, in_maps, core_ids=list(range(N_CORES)), trace=trace
    )
    out = np.empty((B, T, E), dtype=np.float32)
    for c in range(N_CORES):
        out[:, 64 * c:64 * (c + 1), :] = res.results[c]["out"]
    return out, res


def kernel(**inputs):
    out, _ = run(inputs, trace=False)
    return out


# revision 29
# speedup vs baseline: 1.2615x; 1.2615x over previous
"""Trainium2 Bass kernel for nn_MultiHeadAttention_867583393876.

Math (per batch b, head h, all matrices 512x512):
  Qm = x[b] @ WQ[h]; Km = x[b] @ WK[h]; Vm = x[b] @ WV[h]
  S  = Qm @ Km                      (the reference's K.reshape is an identity
                                     on a square matrix, so S = Q @ K, not Q@K^T)
  A  = softmax(S / sqrt(512), axis over the QUERY index t (rows of S))
  Zm = A @ Vm
  out[b] = Z.reshape(512, 4096) @ WO   with Z stacked (h, t, e) -> row-scramble:
      out row t' = h*64 + t//8 uses Z_h rows t = 8*(t'%64)+j, j in [0,8).

Key structural fact: head h only feeds output rows [64h, 64h+64) -- the "WO
projection" concatenates head contributions, it does not sum them.  So the
kernel is sharded head-parallel across the 8 cores with NO collectives: core c
computes out[:, 64c:64(c+1), :] for all 16 batches.

Everything is computed in transposed space (partition = channel) so the
softmax reduction runs along the free axis:
  XT = x[b]^T (via PE transpose), QmT = WQ^T @ XT, Km/Vm natural = XT^T @ W,
  ST = Km^T-contract (lhsT=Km natural) vs QmT, softmax per partition row,
  ZmT = lhsT(Vm natural) @ AT, and the WO stage consumes stride-8 free-dim
  slices of ZmT (which exactly realizes the reference's reshape scramble).

All matmuls run in float32r (13-ish mantissa bits, 4x faster than fp32 on the
PE); inputs are rounded to f32r via copy ops as the BIR verifier requires.
"""

import os

import numpy as np

B, T, E, H = 16, 512, 512, 8
N_CORES = 8
SCALE = 1.0 / 22.627416997969522  # 1/sqrt(512)

_CACHE = {}


def _emit(ctx, nc, tc, tile, mybir, aps, precise):
    import concourse.bass as bass
    from concourse.masks import make_identity

    f32 = mybir.dt.float32
    f32r = mybir.dt.float32r
    x, wq, wk, wv, wo, outp = (
        aps["x"], aps["wq"], aps["wk"], aps["wv"], aps["wo"], aps["out"],
    )
    ts = bass.ts

    def pool(name, bufs, space="SBUF"):
        return ctx.enter_context(tc.tile_pool(name=name, bufs=bufs, space=space))

    # SBUF pools (sizes are KB/partition; SBUF is 224KB/partition)
    wb = 1 if precise else 2
    p_wo = pool("wo", 1)          # 64KB  WO in f32r, resident
    p_w = pool("w", 1)            # 24KB  WQ/WK/WV[h] f32r, resident
    p_stage = pool("stage", 2)    # 4KB  fp32 DMA staging (W, WO)
    p_xn = pool("xn", wb)         # 16KB x[b]^T fp32 staging
    p_xt = pool("xt", wb)         # 16KB  XT f32r
    p_q = pool("q", wb)           # 16KB  QmT f32r
    p_k = pool("k", wb)           # 16KB  Km natural f32r
    p_v = pool("v", wb)           # 16KB  Vm natural f32r
    p_at = pool("at", 1)          # 8KB   AT f32r
    p_scr = pool("scr", 1)        # 2KB   exp scratch fp32
    p_zt = pool("zt", 1)          # 16KB  ZmT f32r for a batch pair (b even+odd)
    p_out = pool("ostage", 1)     # 2KB   output staging
    p_small = pool("small", 4)    # small per-partition scalars
    if precise:
        # residual (error-free-transformation) tiles for the Q/K/S chain
        p_we = pool("we", 1)      # 16KB  WQ/WK residuals f32r
        p_xe = pool("xe", 1)      # 8KB   XT residual f32r
        p_qe = pool("qe", 1)      # 8KB   QmT residual f32r
        p_ke = pool("ke", 1)      # 8KB   Km residual f32r

    ps_mm = pool("ps_mm", 4, space="PSUM")
    ps_st = pool("ps_st", 3, space="PSUM")
    ps_wo = pool("ps_wo", 1, space="PSUM")

    sub = mybir.AluOpType.subtract
    xload = {}

    # ---- load + round weights (once) ----
    def load_round(dram_rows, n_tiles, dst_tile, scope, resid_tile=None, i0=0):
        # dram_rows: function i -> DRAM AP of rows (128, 512)
        with nc.named_scope(scope):
            for i in range(i0, i0 + n_tiles):
                st = p_stage.tile([128, 512], f32, tag="stage")
                nc.sync.dma_start(st[:], dram_rows(i))
                nc.any.tensor_copy(dst_tile[:, ts(i, 512)], st[:])
                if resid_tile is not None:
                    nc.vector.tensor_tensor(
                        resid_tile[:, ts(i, 512)], st[:],
                        dst_tile[:, ts(i, 512)].bitcast(f32), op=sub,
                    )

    # x[b]^T load (pre-transposed on host) + round to f32r; called one batch
    # ahead so the DMA + rounding hide under the previous batch's compute.
    def load_x(bb):
        xn = p_xn.tile([128, 4 * 512], f32, tag="xn")
        for i in range(4):
            nc.sync.dma_start(xn[:, ts(i, 512)], x[bb, i * 128:(i + 1) * 128, :])
        xt = p_xt.tile([128, 4 * 512], f32r, tag="xt")
        for i in range(4):
            nc.any.tensor_copy(xt[:, ts(i, 512)], xn[:, ts(i, 512)])
        xload[bb] = (xn, xt)

    wo_r = p_wo.tile([128, 32 * 512], f32r, tag="wo")
    wq_e = wk_e = None
    if precise:
        wq_e = p_we.tile([128, 4 * 512], f32r, tag="wqe")
        wk_e = p_we.tile([128, 4 * 512], f32r, tag="wke")
    wq_r = p_w.tile([128, 4 * 512], f32r, tag="wq")
    load_round(lambda i: wq[i * 128:(i + 1) * 128, :], 4, wq_r, "load_wq", wq_e)
    wk_r = p_w.tile([128, 4 * 512], f32r, tag="wk")
    load_round(lambda i: wk[i * 128:(i + 1) * 128, :], 4, wk_r, "load_wk", wk_e)
    wv_r = p_w.tile([128, 4 * 512], f32r, tag="wv")
    load_round(lambda i: wv[i * 128:(i + 1) * 128, :], 4, wv_r, "load_wv")

    load_x(0)

    zt_state = [None]
    pending = []

    def emit_z(b, vm, at):
        # ---- ZmT = lhsT(Vm natural) @ AT ----
        # The PSUM->SBUF rounding copy scatters straight into the WO-ready
        # scrambled layout: ZS free index = vblk*1024 + j*128 + (half*64+q)
        # where the Z column t = 8q + j and half = b%2.  The WO-stage lhsT
        # is then a plain contiguous 128-wide slice per (j, vblk) k-tile.
        if b % 2 == 0:
            ztp = p_zt.tile([128, 2 * 4 * 512], f32r, tag="zt")
            zt_state[0] = ztp
        zt = zt_state[0]
        half = b % 2
        for vblk in range(4):
            pz = ps_mm.tile([128, 512], f32, tag="mm")
            for m in range(4):
                nc.tensor.matmul(
                    pz[:],
                    vm[:, m * 512 + vblk * 128: m * 512 + vblk * 128 + 128],
                    at[:, ts(m, 512)],
                    start=(m == 0), stop=(m == 3),
                )
            zf = zt[:]
            pf = pz[:]
            dst = bass.AP(
                zf.tensor, zf.offset + vblk * 1024 + half * 64,
                [list(zf.ap[0]), [1, 64], [128, 8]],
            )
            src = bass.AP(
                pf.tensor, pf.offset, [list(pf.ap[0]), [8, 64], [1, 8]]
            )
            nc.vector.tensor_copy(dst, src)

        # ---- WO stage for the (b-1, b) pair ----
        # lhsT free dims: output partitions 0:64 = batch b-1 rows,
        # 64:128 = batch b rows -- a full M=128 matmul per k-tile.
        if b % 2 == 1:
            po = ps_wo.tile([128, 512], f32, tag="wops")
            for kt in range(32):
                j, dblk = kt // 4, kt % 4
                rhs = wo_r[:, ts(kt, 512)]
                lhs = zt[:, dblk * 1024 + j * 128: dblk * 1024 + (j + 1) * 128]
                nc.tensor.matmul(
                    po[:], lhs, rhs,
                    start=(kt == 0), stop=(kt == 31),
                )
            so = p_out.tile([128, 512], f32, tag="so")
            nc.any.tensor_copy(so[:], po[:])
            nc.sync.dma_start(outp[b - 1], so[0:64, :])
            nc.sync.dma_start(outp[b], so[64:128, :])

    load_x(0)

    for b in range(B):
        if b in (0, 1):
            # all 32 WO tiles must be resident before the first WO stage
            # (end of batch 1); split the load to spread the rounding work
            load_round(lambda i: wo[i * 128:(i + 1) * 128, :], 16, wo_r,
                       f"load_wo{b}", i0=16 * b)
        with nc.named_scope(f"batch{b}"):
            if b + 1 < B:
                load_x(b + 1)
            xn, xt = xload.pop(b)
            xe = None
            if precise:
                xe = p_xe.tile([128, 4 * 512], f32r, tag="xe")
                for i in range(4):
                    nc.vector.tensor_tensor(
                        xe[:, ts(i, 512)], xn[:, ts(i, 512)],
                        xt[:, ts(i, 512)].bitcast(f32), op=sub,
                    )

            def mm_acc(ps_tile, pairs):
                for i, (l, r) in enumerate(pairs):
                    nc.tensor.matmul(
                        ps_tile[:], l, r,
                        start=(i == 0), stop=(i == len(pairs) - 1),
                    )

            def col(w, k, blk):
                return w[:, k * 512 + blk * 128: k * 512 + blk * 128 + 128]

            # ---- QmT = WQ^T @ XT ----
            qt = p_q.tile([128, 4 * 512], f32r, tag="q")
            qe = None
            if precise:
                qe = p_qe.tile([128, 4 * 512], f32r, tag="qe")
            for dblk in range(4):
                pq = ps_mm.tile([128, 512], f32, tag="mm")
                pairs = [(col(wq_r, k, dblk), xt[:, ts(k, 512)]) for k in range(4)]
                if precise:
                    pairs += [(col(wq_r, k, dblk), xe[:, ts(k, 512)]) for k in range(4)]
                    pairs += [(col(wq_e, k, dblk), xt[:, ts(k, 512)]) for k in range(4)]
                mm_acc(pq, pairs)
                # fold the 1/sqrt(512) softmax scale into Q's rounding copy so
                # the ST matmul directly produces scaled logits
                nc.any.tensor_scalar_mul(qt[:, ts(dblk, 512)], pq[:], SCALE)
                if precise:
                    nc.vector.scalar_tensor_tensor(
                        qe[:, ts(dblk, 512)], pq[:], SCALE,
                        qt[:, ts(dblk, 512)].bitcast(f32),
                        op0=mybir.AluOpType.mult, op1=sub,
                    )

            # ---- Km natural = XT^T-contract @ WK ----
            km = p_k.tile([128, 4 * 512], f32r, tag="k")
            ke = None
            if precise:
                ke = p_ke.tile([128, 4 * 512], f32r, tag="ke")
            for tblk in range(4):
                pk = ps_mm.tile([128, 512], f32, tag="mm")
                pairs = [(col(xt, k, tblk), wk_r[:, ts(k, 512)]) for k in range(4)]
                if precise:
                    pairs += [(col(xe, k, tblk), wk_r[:, ts(k, 512)]) for k in range(4)]
                    pairs += [(col(xt, k, tblk), wk_e[:, ts(k, 512)]) for k in range(4)]
                mm_acc(pk, pairs)
                nc.any.tensor_copy(km[:, ts(tblk, 512)], pk[:])
                if precise:
                    nc.vector.tensor_tensor(
                        ke[:, ts(tblk, 512)], pk[:],
                        km[:, ts(tblk, 512)].bitcast(f32), op=sub,
                    )

            # ---- Vm natural = XT^T-contract @ WV ----
            vm = p_v.tile([128, 4 * 512], f32r, tag="v")
            for tblk in range(4):
                pv = ps_mm.tile([128, 512], f32, tag="mm")
                mm_acc(pv, [(col(xt, k, tblk), wv_r[:, ts(k, 512)]) for k in range(4)])
                nc.any.tensor_copy(vm[:, ts(tblk, 512)], pv[:])

            # defer Z + WO of the previous batch to here: its softmax
            # inputs are a full batch old, so the PE never stalls on them,
            # and its matmuls cover the latency of this batch's Q/K rounding
            if pending:
                emit_z(*pending.pop())

            # ---- ST = Km^T-contract @ QmT, then softmax along free axis ----
            at = p_at.tile([128, 4 * 512], f32r, tag="at")
            for sblk in range(4):
                pst = ps_st.tile([128, 512], f32, tag="st")
                pairs = [(col(km, m, sblk), qt[:, ts(m, 512)]) for m in range(4)]
                if precise:
                    pairs += [(col(ke, m, sblk), qt[:, ts(m, 512)]) for m in range(4)]
                    pairs += [(col(km, m, sblk), qe[:, ts(m, 512)]) for m in range(4)]
                mm_acc(pst, pairs)
                nmx = p_small.tile([128, 1], f32, tag="nmx")
                nc.vector.tensor_reduce(
                    nmx[:], pst[:], axis=mybir.AxisListType.X,
                    op=mybir.AluOpType.max, negate=True,
                )
                scr = p_scr.tile([128, 512], f32, tag="scr")
                sm = p_small.tile([128, 1], f32, tag="sm")
                nc.scalar.activation(
                    scr[:], pst[:], mybir.ActivationFunctionType.Exp,
                    bias=nmx[:], scale=1.0, accum_out=sm[:],
                )
                rc = p_small.tile([128, 1], f32, tag="rc")
                nc.vector.reciprocal(rc[:], sm[:])
                nc.vector.tensor_scalar_mul(at[:, ts(sblk, 512)], scr[:], rc[:])

            pending.append((b, vm, at))

    emit_z(*pending.pop())


def _build(precise):
    import concourse.bass as bass  # noqa: F401
    import concourse.tile as tile
    from concourse import bacc, mybir

    nc = bacc.Bacc(
        "TRN2",
        target_bir_lowering=False,
        debug=False,
        enable_asserts=False,
        num_devices=N_CORES,
    )
    f32 = mybir.dt.float32
    aps = {
        "x": nc.dram_tensor("x", (B, E, T), f32, kind="ExternalInput").ap(),
        "wq": nc.dram_tensor("wq", (E, E), f32, kind="ExternalInput").ap(),
        "wk": nc.dram_tensor("wk", (E, E), f32, kind="ExternalInput").ap(),
        "wv": nc.dram_tensor("wv", (E, E), f32, kind="ExternalInput").ap(),
        "wo": nc.dram_tensor("wo", (H * E, E), f32, kind="ExternalInput").ap(),
        "out": nc.dram_tensor("out", (B, 64, E), f32, kind="ExternalOutput").ap(),
    }
    from contextlib import ExitStack

    with tile.TileContext(nc) as tc, ExitStack() as ctx:
        _emit(ctx, nc, tc, tile, mybir, aps, precise)
    nc.compile()
    return nc


DEFAULT_PRECISE = False


def _get_nc(precise=None):
    if precise is None:
        precise = DEFAULT_PRECISE
    key = ("nc", bool(precise))
    if key not in _CACHE:
        _CACHE[key] = _build(precise)
    return _CACHE[key]


def run(inputs, trace=False, precise=None):
    from concourse.bass_utils import run_bass_kernel_spmd

    nc = _get_nc(precise)
    x = np.asarray(inputs["x"], dtype=np.float32)
    xT = np.ascontiguousarray(x.transpose(0, 2, 1))
    WQ = np.asarray(inputs["WQ"], dtype=np.float32)
    WK = np.asarray(inputs["WK"], dtype=np.float32)
    WV = np.asarray(inputs["WV"], dtype=np.float32)
    WO = np.ascontiguousarray(np.asarray(inputs["WO"], dtype=np.float32))
    in_maps = [
        {
            "x": xT,
            "wq": np.ascontiguousarray(WQ[c]),
            "wk": np.ascontiguousarray(WK[c]),
            "wv": np.ascontiguousarray(WV[c]),
            "wo": WO,
        }
        for c in range(N_CORES)
    ]
    res = run_bass_kernel_spmd(
        nc, in_maps, core_ids=list(range(N_CORES)), trace=trace
    )
    out = np.empty((B, T, E), dtype=np.float32)
    for c in range(N_CORES):
        out[:, 64 * c:64 * (c + 1), :] = res.results[c]["out"]
    return out, res


def kernel(**inputs):
    out, _ = run(inputs, trace=False)
    return out
